# Initial kernel scaffold
#
"""3-layer GCN (DGL GraphConv, norm='both') on 8 Trainium2 NeuronCores.

Strategy:
  - Nodes are packed into 80 balanced bins (128 slots each) by in-degree
    (greedy least-loaded), 10 bins per core -> 1280 padded rows/core.
  - Edges live with the owner (bin) of their dst node. segment_sum is done
    as one-hot "scatter matmuls" on the TensorEngine: for each dst block,
    agg[128d, D] += S_kt[128e, 128d].T @ msg_kt[128e, D], where msg rows are
    fetched with dma_gather (SWDGE) and S is a host-built one-hot matrix
    carrying the edge weights norm_src[src]*norm_dst[dst].
  - Dense W matmuls run per dst block: PE-transpose agg -> aggT, then
    x = aggT.T @ W (+ bias via K=1 matmul) with ReLU fused into the
    PSUM->SBUF copy. Matmuls use float32r (~1 cycle/row at N>=512).
  - Layer outputs are exchanged with an ncfw AllGather so every core can
    gather any source row for the next layer's SpMM.
  - Layer 3 computes y3 = x3 @ W3 locally first (64 wide), AllGathers the
    small y3, then aggregates: A (x W3) == (A x) W3.
"""
import sys
sys.path.insert(0, '/opt/trn_rl_repo')
import numpy as np

N_CORES = 8


def _ag_splits(nblk):
    """Block-index boundaries of the staged AllGather slabs."""
    if nblk <= 2:
        return [0, nblk]
    fr = [0, round(0.3 * nblk), round(0.6 * nblk), round(0.8 * nblk),
          nblk - 1, nblk]
    return sorted(set(b for b in fr if 0 <= b <= nblk))


# ---------------------------------------------------------------- host prep
def _partition_nodes(deg_in, n_nodes, nbins):
    """Greedy balanced-edge binning: nodes (sorted by in-degree desc) go to
    the least-loaded bin with a free slot (capacity 128)."""
    import heapq
    order = np.argsort(-deg_in, kind="stable")
    heap = [(0, b) for b in range(nbins)]
    heapq.heapify(heap)
    bin_of = np.empty(n_nodes, np.int32)
    slot_of = np.empty(n_nodes, np.int32)
    count = np.zeros(nbins, np.int64)
    load = np.zeros(nbins, np.int64)
    for n in order:
        while True:
            l, b = heapq.heappop(heap)
            if count[b] < 128:
                break
            # full bin: drop from heap permanently
        bin_of[n] = b
        slot_of[n] = count[b]
        count[b] += 1
        load[b] += int(deg_in[n])
        heapq.heappush(heap, (l + int(deg_in[n]), b))
    return bin_of, slot_of, load


def _prep(h, src, dst, cfg):
    """Build per-core S one-hot tiles, gather indices, and row maps."""
    N, E, NBLK = cfg["N"], cfg["E"], cfg["NBLK"]
    nbins = N_CORES * NBLK
    deg_out = np.bincount(src, minlength=N)
    deg_in = np.bincount(dst, minlength=N)
    norm_src = np.clip(deg_out, 1, None).astype(np.float32) ** np.float32(-0.5)
    norm_dst = np.clip(deg_in, 1, None).astype(np.float32) ** np.float32(-0.5)
    w = (norm_src[src] * norm_dst[dst]).astype(np.float32)

    bin_of, slot_of, load = _partition_nodes(deg_in, N, nbins)

    # deal bins to cores snake-wise by load to balance core totals
    order = np.argsort(-load, kind="stable")
    core_of_bin = np.empty(nbins, np.int32)
    blk_of_bin = np.empty(nbins, np.int32)
    nextblk = [0] * N_CORES
    for i, b in enumerate(order):
        r = i // N_CORES
        c = (i % N_CORES) if r % 2 == 0 else (N_CORES - 1 - (i % N_CORES))
        core_of_bin[b] = c
        blk_of_bin[b] = nextblk[c]
        nextblk[c] += 1

    RPC = NBLK * 128
    row_of_node = (core_of_bin[bin_of] * RPC + blk_of_bin[bin_of] * 128
                   + slot_of).astype(np.int32)
    # gather-id layout after the staged slab AllGathers: slab q holds rows
    # [b_q, e_q) of every core, concatenated core-major at offset 8*b_q
    sp = np.array(_ag_splits(NBLK)) * 128
    _c = row_of_node // RPC
    _r = row_of_node % RPC
    _q = np.searchsorted(sp, _r, side="right") - 1
    gid_of_node = (N_CORES * sp[_q] + _c * (sp[_q + 1] - sp[_q])
                   + _r - sp[_q]).astype(np.int32)

    # group edges by dst bin
    ebin = bin_of[dst]
    eorder = np.argsort(ebin, kind="stable")
    counts = np.bincount(ebin, minlength=nbins)
    kt_blk = max(cfg["KT_MIN"], int(-(-counts.max() // 128)))
    kt_blk = -(-kt_blk // 4) * 4          # multiple of the 4-ktile chunk
    kt_tot = NBLK * kt_blk

    idx1 = np.zeros((N_CORES, kt_tot * 128), np.int16)
    idx23 = np.zeros((N_CORES, kt_tot * 128), np.int16)
    S = np.zeros((N_CORES, 128, kt_tot, 128), np.float32)
    bounds = np.concatenate([[0], np.cumsum(counts)])
    for b in range(nbins):
        es = eorder[bounds[b]:bounds[b + 1]]
        c, blk = int(core_of_bin[b]), int(blk_of_bin[b])
        p = np.arange(len(es))
        kt = blk * kt_blk + p // 128
        esl = p % 128
        gpos = blk * kt_blk * 128 + p
        idx1[c, gpos] = src[es].astype(np.int16)
        idx23[c, gpos] = gid_of_node[src[es]].astype(np.int16)
        S[c, esl, kt, slot_of[dst[es]]] = w[es]

    def wrap(ix):  # -> [128, kt_tot*8] wrapped for the 8 Q7 cores
        return np.tile(ix.reshape(-1, 16).T, (8, 1)).copy()

    idx1_w = np.stack([wrap(idx1[c]) for c in range(N_CORES)])
    idx23_w = np.stack([wrap(idx23[c]) for c in range(N_CORES)])
    return dict(S=S, idx1=idx1_w, idx23=idx23_w, row_of_node=row_of_node,
                kt_blk=kt_blk, kt_tot=kt_tot)


# ---------------------------------------------------------------- device prog
def _build(cfg, kt_blk, use_bias):
    import concourse.bacc as bacc
    import concourse.mybir as mybir
    import concourse.tile as tile
    from concourse.library_config import mlp

    f32 = mybir.dt.float32
    f32r = mybir.dt.float32r
    i16 = mybir.dt.int16
    RELU = mybir.ActivationFunctionType.Relu
    COPY = mybir.ActivationFunctionType.Copy

    N, D, C, NBLK = cfg["N"], cfg["D"], cfg["C"], cfg["NBLK"]
    RPC = NBLK * 128
    NPAD = N_CORES * RPC
    KT = kt_blk
    KT_TOT = NBLK * KT
    CH = 2                      # k-tiles per gather chunk (256 rows)
    CH3 = min(8, kt_blk)        # k-tiles per layer-3 gather chunk
    KD = D // 128               # dense contraction k-tiles
    ND = 512 if D % 512 == 0 else D
    NT = D // ND                # dense n-tiles
    TPW = min(512, D)           # transposes packed per tps tile
    TPG = TPW // 128
    SPL = _ag_splits(NBLK)

    nc = bacc.Bacc("TRN2", target_bir_lowering=False, debug=False,
                   num_devices=N_CORES, num_swdge_queues=4,
                   dynamic_dma_scratch_size=32768)

    hx = nc.dram_tensor("hx", [N, D], f32, kind="ExternalInput")
    sker = nc.dram_tensor("sker", [128, KT_TOT, 128], f32, kind="ExternalInput")
    idx1_h = nc.dram_tensor("idx1", [128, KT_TOT * 8], i16, kind="ExternalInput")
    idx23_h = nc.dram_tensor("idx23", [128, KT_TOT * 8], i16, kind="ExternalInput")
    w12_h = nc.dram_tensor("w12", [2, 128, KD, D], f32, kind="ExternalInput")
    w3_h = nc.dram_tensor("w3", [128, KD, C], f32, kind="ExternalInput")
    ident_h = nc.dram_tensor("ident", [128, 128], f32, kind="ExternalInput")
    bias_h = nc.dram_tensor("biases", [1, 2 * D + C + 128], f32, kind="ExternalInput")
    out_h = nc.dram_tensor("out", [RPC, C], f32, kind="ExternalOutput")

    ag_in = nc.dram_tensor("ag_in", [RPC, D], f32, kind="Internal")
    ag_out = nc.dram_tensor("ag_out", [NPAD, D], f32, kind="Internal",
                            addr_space="Shared")
    ag3_in = nc.dram_tensor("ag3_in", [RPC, C], f32, kind="Internal")
    ag3_out = nc.dram_tensor("ag3_out", [NPAD, C], f32, kind="Internal",
                             addr_space="Shared")

    with tile.TileContext(nc) as tc:
        nc.gpsimd.load_library(mlp)
        with (
            tc.tile_pool(name="const", bufs=1) as cp,
            tc.tile_pool(name="msg", bufs=3) as mp,
            tc.tile_pool(name="msg3", bufs=2) as mp3,
            tc.tile_pool(name="work", bufs=2) as wp,
            tc.tile_pool(name="aggps", bufs=2, space="PSUM") as aps,
            tc.tile_pool(name="densps", bufs=2, space="PSUM") as dps,
            tc.tile_pool(name="tpsps", bufs=2, space="PSUM") as tps,
        ):
            idx1_t = cp.tile([128, KT_TOT * 8], i16, tag="idx1")
            nc.sync.dma_start(idx1_t[:], idx1_h[:])
            s_blk = []
            for b in range(NBLK):
                sb = cp.tile([128, KT, 128], f32r, tag=f"s{b}")
                nc.sync.dma_start(sb[:], sker[:, b * KT:(b + 1) * KT, :]
                                  .bitcast(f32r))
                s_blk.append(sb)
            idx23_t = cp.tile([128, KT_TOT * 8], i16, tag="idx23")
            nc.sync.dma_start(idx23_t[:], idx23_h[:])
            w_t = cp.tile([128, KD, D], f32r, tag="w")
            nc.sync.dma_start(w_t[:], w12_h[0].bitcast(f32r))
            w3_t = cp.tile([128, KD, C], f32r, tag="w3")
            nc.sync.dma_start(w3_t[:], w3_h[:].bitcast(f32r))
            ident_t = cp.tile([128, 128], f32, tag="ident")
            nc.sync.dma_start(ident_t[:], ident_h[:])
            if use_bias:
                brow_t = cp.tile([1, 2 * D + C + 128], f32r, tag="brow")
                nc.sync.dma_start(brow_t[:], bias_h[:].bitcast(f32r))
                ones_t = brow_t[:, 2 * D + C:2 * D + C + 128]

            qctr = [0]

            def spmm_block(b, src_ap, idx_t, width, ch, msg_pool, psum_pool,
                           close=True):
                """agg[128, width] for dst block b via gather + one-hot MMs."""
                agg = psum_pool.tile([128, width], f32, tag="aggps")
                nspl = max(1, width // 512)
                for c in range(KT // ch):
                    msg = msg_pool.tile([128, ch, width], f32r, tag="m")
                    col0 = (b * KT + c * ch) * 8
                    q = qctr[0] % 4
                    qctr[0] += 1
                    nc.gpsimd.dma_gather(
                        msg[:], src_ap, idx_t[:, col0:col0 + ch * 8],
                        ch * 128, ch * 128, width, queue_num=q)
                    for k in range(ch):
                        kt = b * KT + c * ch + k
                        first = (c == 0 and k == 0)
                        last = (c == KT // ch - 1 and k == ch - 1)
                        for n in range(nspl):
                            w0 = n * (width // nspl)
                            w1 = (n + 1) * (width // nspl)
                            nc.tensor.matmul(
                                agg[:, w0:w1], s_blk[b][:, kt - b * KT, :],
                                msg[:, k, w0:w1],
                                start=first, stop=last and close)
                return agg

            def transpose_to(dst_t, src_sb):
                """dst_t[128, KD, 128] (f32r) = src_sb[128, D] transposed."""
                for g in range(KD // TPG):
                    tp = tps.tile([128, TPW], f32, tag="tp")
                    for j in range(TPG):
                        col = (g * TPG + j) * 128
                        nc.tensor.transpose(
                            tp[:, j * 128:(j + 1) * 128],
                            src_sb[:, col:col + 128], ident_t[:])
                    nc.vector.tensor_copy(
                        dst_t[:, g * TPG:(g + 1) * TPG, :].rearrange(
                            "p a b -> p (a b)"), tp[:])

            def dense_block(aggT_t, out_sb, bias_off, relu):
                """out_sb[128, D] = act(aggT.T @ W + b)."""
                for n in range(NT):
                    dp = dps.tile([128, ND], f32, tag="dp")
                    for k in range(KD):
                        nc.tensor.matmul(
                            dp[:], aggT_t[:, k, :], w_t[:, k, n * ND:(n + 1) * ND],
                            start=(k == 0), stop=(k == KD - 1 and not use_bias))
                    if use_bias:
                        nc.tensor.matmul(
                            dp[:], ones_t,
                            brow_t[:, bias_off + n * ND:bias_off + (n + 1) * ND],
                            start=False, stop=True)
                    nc.scalar.activation(out_sb[:, n * ND:(n + 1) * ND], dp[:],
                                         RELU if relu else COPY)

            # ---------------- layer 1 + 2
            for layer in range(2):
                src_ap = (hx[:] if layer == 0 else ag_out[:]).bitcast(f32r)
                idx_t = idx1_t if layer == 0 else idx23_t
                for b in range(NBLK):
                    agg = spmm_block(b, src_ap, idx_t, D, CH, mp, aps)
                    agg_sb = wp.tile([128, D], f32, tag="aggsb")
                    nc.scalar.activation(agg_sb[:], agg[:], COPY)
                    aggT_t = wp.tile([128, KD, 128], f32r, tag="aggT")
                    transpose_to(aggT_t, agg_sb)
                    x_sb = wp.tile([128, D], f32, tag="x")
                    dense_block(aggT_t, x_sb, layer * D, relu=True)
                    if layer == 0:
                        nc.sync.dma_start(ag_in[b * 128:(b + 1) * 128, :], x_sb[:])
                        if b + 1 in SPL[1:]:
                            r0, r1 = SPL[SPL.index(b + 1) - 1] * 128, (b + 1) * 128
                            nc.gpsimd.collective_compute(
                                "AllGather", mybir.AluOpType.bypass,
                                ins=[ag_in[r0:r1, :]],
                                outs=[ag_out[N_CORES * r0:N_CORES * r1, :]],
                                replica_groups=[list(range(N_CORES))])
                    else:
                        # y3 = x3 @ W3 for this block
                        x3T_t = wp.tile([128, KD, 128], f32r, tag="x3T")
                        transpose_to(x3T_t, x_sb)
                        yp = dps.tile([128, C], f32, tag="dp")
                        for k in range(KD):
                            nc.tensor.matmul(yp[:], x3T_t[:, k, :], w3_t[:, k, :],
                                             start=(k == 0), stop=(k == KD - 1))
                        y_sb = wp.tile([128, C], f32, tag="y")
                        nc.scalar.activation(y_sb[:], yp[:], COPY)
                        nc.sync.dma_start(ag3_in[b * 128:(b + 1) * 128, :], y_sb[:])
                        if b + 1 in SPL[1:]:
                            r0, r1 = SPL[SPL.index(b + 1) - 1] * 128, (b + 1) * 128
                            nc.gpsimd.collective_compute(
                                "AllGather", mybir.AluOpType.bypass,
                                ins=[ag3_in[r0:r1, :]],
                                outs=[ag3_out[N_CORES * r0:N_CORES * r1, :]],
                                replica_groups=[list(range(N_CORES))])
                if layer == 0:
                    nc.sync.dma_start(w_t[:], w12_h[1].bitcast(f32r))

            # ---------------- layer 3: out = A y3 (+ b3)
            for b in range(NBLK):
                agg3 = spmm_block(b, ag3_out[:].bitcast(f32r), idx23_t, C,
                                  CH3, mp3, aps, close=not use_bias)
                if use_bias:
                    nc.tensor.matmul(agg3[:], ones_t,
                                     brow_t[:, 2 * D:2 * D + C],
                                     start=False, stop=True)
                o_sb = wp.tile([128, C], f32, tag="o")
                nc.scalar.activation(o_sb[:], agg3[:], COPY)
                nc.sync.dma_start(out_h[b * 128:(b + 1) * 128, :], o_sb[:])

    nc.compile()
    return nc


_CACHE = {}


def _get_prog(cfg, kt_blk, use_bias):
    key = (cfg["N"], cfg["D"], kt_blk, use_bias)
    if key not in _CACHE:
        _CACHE[key] = _build(cfg, kt_blk, use_bias)
    return _CACHE[key]


# ---------------------------------------------------------------- entry point
CFG_FULL = dict(N=10000, E=160000, D=1024, C=64, NBLK=10, KT_MIN=16)


def kernel(h, src, dst, W1, b1, W2, b2, W3, b3, cfg=CFG_FULL):
    from concourse.bass_utils import run_bass_kernel_spmd

    h = np.asarray(h, np.float32)
    src = np.asarray(src, np.int32)
    dst = np.asarray(dst, np.int32)
    N, D, C, NBLK = cfg["N"], cfg["D"], cfg["C"], cfg["NBLK"]
    RPC = NBLK * 128
    KD = D // 128

    pp = _prep(h, src, dst, cfg)
    use_bias = bool(np.any(b1) or np.any(b2) or np.any(b3))
    nc = _get_prog(cfg, pp["kt_blk"], use_bias)

    w12 = np.stack([
        np.asarray(W1, np.float32).reshape(KD, 128, D).transpose(1, 0, 2),
        np.asarray(W2, np.float32).reshape(KD, 128, D).transpose(1, 0, 2)])
    w3 = np.asarray(W3, np.float32).reshape(KD, 128, C).transpose(1, 0, 2)
    biases = np.concatenate([np.asarray(b1, np.float32),
                             np.asarray(b2, np.float32),
                             np.asarray(b3, np.float32),
                             np.ones(128, np.float32)])[None, :]
    ident = np.eye(128, dtype=np.float32)

    in_maps = [
        dict(hx=h, sker=np.ascontiguousarray(pp["S"][c]),
             idx1=pp["idx1"][c], idx23=pp["idx23"][c],
             w12=w12, w3=w3, ident=ident, biases=biases)
        for c in range(N_CORES)
    ]
    res = run_bass_kernel_spmd(nc, in_maps, core_ids=list(range(N_CORES)))

    out = np.zeros((N, C), np.float32)
    rows = pp["row_of_node"]
    allout = np.concatenate([res.results[c]["out"] for c in range(N_CORES)],
                            axis=0)
    out[:, :] = allout[rows]
    return out



# revision 3
# speedup vs baseline: 1.1574x; 1.1574x over previous
"""3-layer GCN (DGL GraphConv, norm='both') on 8 Trainium2 NeuronCores.

Strategy:
  - Nodes are packed into 80 balanced bins (128 slots each) by in-degree
    (greedy least-loaded), 10 bins per core -> 1280 padded rows/core.
  - Edges live with the owner (bin) of their dst node. segment_sum is done
    as one-hot "scatter matmuls" on the TensorEngine: for each dst block,
    agg[128d, D] += S_kt[128e, 128d].T @ msg_kt[128e, D], where msg rows are
    fetched with dma_gather (SWDGE) and S is a host-built one-hot matrix
    carrying the edge weights norm_src[src]*norm_dst[dst].
  - All feature traffic (h, layer activations, S, W) is bf16: halves both
    the per-edge gather DMA and the AllGather wire bytes vs f32. PSUM
    accumulation stays f32.
  - Dense W matmuls run per dst block: PE-transpose agg -> aggT, then
    x = aggT.T @ W (+ bias via K=1 matmul) with ReLU fused into the
    PSUM->SBUF copy.
  - Layer outputs are exchanged with an ncfw AllGather so every core can
    gather any source row for the next layer's SpMM.
  - Layer 3 computes y3 = x3 @ W3 locally first (padded to 128 wide so the
    bf16 gather rows stay 256B-aligned), AllGathers the small y3, then
    aggregates: A (x W3) == (A x) W3.
"""
import sys
sys.path.insert(0, '/opt/trn_rl_repo')
import numpy as np
import ml_dtypes

BF16 = ml_dtypes.bfloat16
N_CORES = 8


def _ag_splits(nblk):
    """Block-index boundaries of the staged AllGather slabs."""
    if nblk <= 2:
        return [0, nblk]
    fr = [0, round(0.3 * nblk), round(0.6 * nblk), round(0.8 * nblk),
          nblk - 1, nblk]
    return sorted(set(b for b in fr if 0 <= b <= nblk))


# ---------------------------------------------------------------- host prep
def _partition_nodes(deg_in, n_nodes, nbins):
    """Greedy balanced-edge binning: nodes (sorted by in-degree desc) go to
    the least-loaded bin with a free slot (capacity 128)."""
    import heapq
    order = np.argsort(-deg_in, kind="stable")
    heap = [(0, b) for b in range(nbins)]
    heapq.heapify(heap)
    bin_of = np.empty(n_nodes, np.int32)
    slot_of = np.empty(n_nodes, np.int32)
    count = np.zeros(nbins, np.int64)
    load = np.zeros(nbins, np.int64)
    for n in order:
        while True:
            l, b = heapq.heappop(heap)
            if count[b] < 128:
                break
            # full bin: drop from heap permanently
        bin_of[n] = b
        slot_of[n] = count[b]
        count[b] += 1
        load[b] += int(deg_in[n])
        heapq.heappush(heap, (l + int(deg_in[n]), b))
    return bin_of, slot_of, load


def _prep(h, src, dst, cfg):
    """Build per-core S one-hot tiles, gather indices, and row maps."""
    N, E, NBLK = cfg["N"], cfg["E"], cfg["NBLK"]
    nbins = N_CORES * NBLK
    deg_out = np.bincount(src, minlength=N)
    deg_in = np.bincount(dst, minlength=N)
    norm_src = np.clip(deg_out, 1, None).astype(np.float32) ** np.float32(-0.5)
    norm_dst = np.clip(deg_in, 1, None).astype(np.float32) ** np.float32(-0.5)
    w = (norm_src[src] * norm_dst[dst]).astype(np.float32)

    bin_of, slot_of, load = _partition_nodes(deg_in, N, nbins)

    # deal bins to cores snake-wise by load to balance core totals
    order = np.argsort(-load, kind="stable")
    core_of_bin = np.empty(nbins, np.int32)
    blk_of_bin = np.empty(nbins, np.int32)
    nextblk = [0] * N_CORES
    for i, b in enumerate(order):
        r = i // N_CORES
        c = (i % N_CORES) if r % 2 == 0 else (N_CORES - 1 - (i % N_CORES))
        core_of_bin[b] = c
        blk_of_bin[b] = nextblk[c]
        nextblk[c] += 1

    RPC = NBLK * 128
    row_of_node = (core_of_bin[bin_of] * RPC + blk_of_bin[bin_of] * 128
                   + slot_of).astype(np.int32)
    # gather-id layout after the staged slab AllGathers: slab q holds rows
    # [b_q, e_q) of every core, concatenated core-major at offset 8*b_q
    sp = np.array(_ag_splits(NBLK)) * 128
    _c = row_of_node // RPC
    _r = row_of_node % RPC
    _q = np.searchsorted(sp, _r, side="right") - 1
    gid_of_node = (N_CORES * sp[_q] + _c * (sp[_q + 1] - sp[_q])
                   + _r - sp[_q]).astype(np.int32)

    # group edges by dst bin
    ebin = bin_of[dst]
    eorder = np.argsort(ebin, kind="stable")
    counts = np.bincount(ebin, minlength=nbins)
    kt_blk = max(cfg["KT_MIN"], int(-(-counts.max() // 128)))
    kt_blk = -(-kt_blk // 4) * 4          # multiple of the 4-ktile chunk
    kt_tot = NBLK * kt_blk

    idx1 = np.zeros((N_CORES, kt_tot * 128), np.int16)
    idx23 = np.zeros((N_CORES, kt_tot * 128), np.int16)
    S = np.zeros((N_CORES, 128, kt_tot, 128), np.float32)
    bounds = np.concatenate([[0], np.cumsum(counts)])
    for b in range(nbins):
        es = eorder[bounds[b]:bounds[b + 1]]
        c, blk = int(core_of_bin[b]), int(blk_of_bin[b])
        p = np.arange(len(es))
        kt = blk * kt_blk + p // 128
        esl = p % 128
        gpos = blk * kt_blk * 128 + p
        idx1[c, gpos] = src[es].astype(np.int16)
        idx23[c, gpos] = gid_of_node[src[es]].astype(np.int16)
        S[c, esl, kt, slot_of[dst[es]]] = w[es]

    def wrap(ix):  # -> [128, kt_tot*8] wrapped for the 8 Q7 cores
        return np.tile(ix.reshape(-1, 16).T, (8, 1)).copy()

    idx1_w = np.stack([wrap(idx1[c]) for c in range(N_CORES)])
    idx23_w = np.stack([wrap(idx23[c]) for c in range(N_CORES)])
    return dict(S=S, idx1=idx1_w, idx23=idx23_w, row_of_node=row_of_node,
                kt_blk=kt_blk, kt_tot=kt_tot)


# ---------------------------------------------------------------- device prog
def _build(cfg, kt_blk, use_bias):
    import concourse.bacc as bacc
    import concourse.mybir as mybir
    import concourse.tile as tile
    from concourse.library_config import mlp

    f32 = mybir.dt.float32
    bf16 = mybir.dt.bfloat16
    i16 = mybir.dt.int16
    RELU = mybir.ActivationFunctionType.Relu
    COPY = mybir.ActivationFunctionType.Copy

    N, D, C, NBLK = cfg["N"], cfg["D"], cfg["C"], cfg["NBLK"]
    CP = 128                    # padded layer-3 width (bf16 gather: 256B rows)
    RPC = NBLK * 128
    NPAD = N_CORES * RPC
    KT = kt_blk
    KT_TOT = NBLK * KT
    CH = 4                      # k-tiles per gather chunk (512 rows)
    CH3 = min(8, kt_blk)        # k-tiles per layer-3 gather chunk
    KD = D // 128               # dense contraction k-tiles
    ND = 512 if D % 512 == 0 else D
    NT = D // ND                # dense n-tiles
    TPW = min(512, D)           # transposes packed per tps tile
    TPG = TPW // 128
    SPL = _ag_splits(NBLK)

    nc = bacc.Bacc("TRN2", target_bir_lowering=False, debug=False,
                   num_devices=N_CORES, num_swdge_queues=4,
                   dynamic_dma_scratch_size=32768)

    hx = nc.dram_tensor("hx", [N, D], bf16, kind="ExternalInput")
    sker = nc.dram_tensor("sker", [128, KT_TOT, 128], bf16, kind="ExternalInput")
    idx1_h = nc.dram_tensor("idx1", [128, KT_TOT * 8], i16, kind="ExternalInput")
    idx23_h = nc.dram_tensor("idx23", [128, KT_TOT * 8], i16, kind="ExternalInput")
    w12_h = nc.dram_tensor("w12", [2, 128, KD, D], bf16, kind="ExternalInput")
    w3_h = nc.dram_tensor("w3", [128, KD, CP], bf16, kind="ExternalInput")
    ident_h = nc.dram_tensor("ident", [128, 128], bf16, kind="ExternalInput")
    bias_h = nc.dram_tensor("biases", [1, 2 * D + CP + 128], bf16,
                            kind="ExternalInput")
    out_h = nc.dram_tensor("out", [RPC, C], f32, kind="ExternalOutput")

    ag_in = nc.dram_tensor("ag_in", [RPC, D], bf16, kind="Internal")
    ag_out = nc.dram_tensor("ag_out", [NPAD, D], bf16, kind="Internal",
                            addr_space="Shared")
    ag3_in = nc.dram_tensor("ag3_in", [RPC, CP], bf16, kind="Internal")
    ag3_out = nc.dram_tensor("ag3_out", [NPAD, CP], bf16, kind="Internal",
                             addr_space="Shared")

    with tile.TileContext(nc) as tc:
        nc.gpsimd.load_library(mlp)
        with (
            tc.tile_pool(name="const", bufs=1) as cp,
            tc.tile_pool(name="msg", bufs=3) as mp,
            tc.tile_pool(name="msg3", bufs=2) as mp3,
            tc.tile_pool(name="work", bufs=2) as wp,
            tc.tile_pool(name="aggps", bufs=2, space="PSUM") as aps,
            tc.tile_pool(name="densps", bufs=2, space="PSUM") as dps,
            tc.tile_pool(name="tpsps", bufs=2, space="PSUM") as tps,
        ):
            idx1_t = cp.tile([128, KT_TOT * 8], i16, tag="idx1")
            nc.sync.dma_start(idx1_t[:], idx1_h[:])
            s_blk = []
            for b in range(NBLK):
                sb = cp.tile([128, KT, 128], bf16, tag=f"s{b}")
                nc.sync.dma_start(sb[:], sker[:, b * KT:(b + 1) * KT, :])
                s_blk.append(sb)
            idx23_t = cp.tile([128, KT_TOT * 8], i16, tag="idx23")
            nc.sync.dma_start(idx23_t[:], idx23_h[:])
            w_t = cp.tile([128, KD, D], bf16, tag="w")
            nc.sync.dma_start(w_t[:], w12_h[0])
            w3_t = cp.tile([128, KD, CP], bf16, tag="w3")
            nc.sync.dma_start(w3_t[:], w3_h[:])
            ident_t = cp.tile([128, 128], bf16, tag="ident")
            nc.sync.dma_start(ident_t[:], ident_h[:])
            if use_bias:
                brow_t = cp.tile([1, 2 * D + CP + 128], bf16, tag="brow")
                nc.sync.dma_start(brow_t[:], bias_h[:])
                ones_t = brow_t[:, 2 * D + CP:2 * D + CP + 128]

            qctr = [0]

            def spmm_block(b, src_ap, idx_t, width, ch, msg_pool, psum_pool,
                           close=True):
                """agg[128, width] for dst block b via gather + one-hot MMs."""
                agg = psum_pool.tile([128, width], f32, tag="aggps")
                nspl = max(1, width // 512)
                for c in range(KT // ch):
                    msg = msg_pool.tile([128, ch, width], bf16, tag="m")
                    col0 = (b * KT + c * ch) * 8
                    q = qctr[0] % 4
                    qctr[0] += 1
                    nc.gpsimd.dma_gather(
                        msg[:], src_ap, idx_t[:, col0:col0 + ch * 8],
                        ch * 128, ch * 128, width, queue_num=q)
                    for k in range(ch):
                        kt = b * KT + c * ch + k
                        first = (c == 0 and k == 0)
                        last = (c == KT // ch - 1 and k == ch - 1)
                        for n in range(nspl):
                            w0 = n * (width // nspl)
                            w1 = (n + 1) * (width // nspl)
                            nc.tensor.matmul(
                                agg[:, w0:w1], s_blk[b][:, kt - b * KT, :],
                                msg[:, k, w0:w1],
                                start=first, stop=last and close)
                return agg

            def transpose_to(dst_t, src_sb):
                """dst_t[128, KD, 128] (bf16) = src_sb[128, D] transposed."""
                for g in range(KD // TPG):
                    tp = tps.tile([128, TPW], bf16, tag="tp")
                    for j in range(TPG):
                        col = (g * TPG + j) * 128
                        nc.tensor.transpose(
                            tp[:, j * 128:(j + 1) * 128],
                            src_sb[:, col:col + 128], ident_t[:])
                    nc.vector.tensor_copy(
                        dst_t[:, g * TPG:(g + 1) * TPG, :].rearrange(
                            "p a b -> p (a b)"), tp[:])

            def dense_block(aggT_t, out_sb, bias_off, relu):
                """out_sb[128, D] = act(aggT.T @ W + b)."""
                for n in range(NT):
                    dp = dps.tile([128, ND], f32, tag="dp")
                    for k in range(KD):
                        nc.tensor.matmul(
                            dp[:], aggT_t[:, k, :], w_t[:, k, n * ND:(n + 1) * ND],
                            start=(k == 0), stop=(k == KD - 1 and not use_bias))
                    if use_bias:
                        nc.tensor.matmul(
                            dp[:], ones_t,
                            brow_t[:, bias_off + n * ND:bias_off + (n + 1) * ND],
                            start=False, stop=True)
                    nc.scalar.activation(out_sb[:, n * ND:(n + 1) * ND], dp[:],
                                         RELU if relu else COPY)

            # ---------------- layer 1 + 2
            for layer in range(2):
                src_ap = hx[:] if layer == 0 else ag_out[:]
                idx_t = idx1_t if layer == 0 else idx23_t
                for b in range(NBLK):
                    agg = spmm_block(b, src_ap, idx_t, D, CH, mp, aps)
                    agg_sb = wp.tile([128, D], bf16, tag="aggsb")
                    nc.scalar.activation(agg_sb[:], agg[:], COPY)
                    aggT_t = wp.tile([128, KD, 128], bf16, tag="aggT")
                    transpose_to(aggT_t, agg_sb)
                    x_sb = wp.tile([128, D], bf16, tag="x")
                    dense_block(aggT_t, x_sb, layer * D, relu=True)
                    if layer == 0:
                        nc.sync.dma_start(ag_in[b * 128:(b + 1) * 128, :], x_sb[:])
                        if b + 1 in SPL[1:]:
                            r0, r1 = SPL[SPL.index(b + 1) - 1] * 128, (b + 1) * 128
                            nc.gpsimd.collective_compute(
                                "AllGather", mybir.AluOpType.bypass,
                                ins=[ag_in[r0:r1, :]],
                                outs=[ag_out[N_CORES * r0:N_CORES * r1, :]],
                                replica_groups=[list(range(N_CORES))])
                    else:
                        # y3 = x3 @ W3 for this block (CP-wide, zero-padded)
                        x3T_t = wp.tile([128, KD, 128], bf16, tag="x3T")
                        transpose_to(x3T_t, x_sb)
                        yp = dps.tile([128, CP], f32, tag="dp")
                        for k in range(KD):
                            nc.tensor.matmul(yp[:], x3T_t[:, k, :], w3_t[:, k, :],
                                             start=(k == 0), stop=(k == KD - 1))
                        y_sb = wp.tile([128, CP], bf16, tag="y")
                        nc.scalar.activation(y_sb[:], yp[:], COPY)
                        nc.sync.dma_start(ag3_in[b * 128:(b + 1) * 128, :], y_sb[:])
                        if b + 1 in SPL[1:]:
                            r0, r1 = SPL[SPL.index(b + 1) - 1] * 128, (b + 1) * 128
                            nc.gpsimd.collective_compute(
                                "AllGather", mybir.AluOpType.bypass,
                                ins=[ag3_in[r0:r1, :]],
                                outs=[ag3_out[N_CORES * r0:N_CORES * r1, :]],
                                replica_groups=[list(range(N_CORES))])
                if layer == 0:
                    nc.sync.dma_start(w_t[:], w12_h[1])

            # ---------------- layer 3: out = A y3 (+ b3)
            for b in range(NBLK):
                agg3 = spmm_block(b, ag3_out[:], idx23_t, CP,
                                  CH3, mp3, aps, close=not use_bias)
                if use_bias:
                    nc.tensor.matmul(agg3[:], ones_t,
                                     brow_t[:, 2 * D:2 * D + CP],
                                     start=False, stop=True)
                o_sb = wp.tile([128, C], f32, tag="o")
                nc.scalar.activation(o_sb[:], agg3[:, :C], COPY)
                nc.sync.dma_start(out_h[b * 128:(b + 1) * 128, :], o_sb[:])

    nc.compile()
    return nc


_CACHE = {}
_LAST_NC = [None]


def _get_prog(cfg, kt_blk, use_bias):
    key = (cfg["N"], cfg["D"], kt_blk, use_bias)
    if key not in _CACHE:
        _CACHE[key] = _build(cfg, kt_blk, use_bias)
    _LAST_NC[0] = _CACHE[key]
    return _CACHE[key]


# ---------------------------------------------------------------- entry point
CFG_FULL = dict(N=10000, E=160000, D=1024, C=64, NBLK=10, KT_MIN=16)

_PREP_CACHE = {}


def _in_maps(ins, cfg=CFG_FULL):
    """Host-side prep: returns (in_maps, prep_dict, nc). Caches on the
    (src, dst) id so the profiled rerun in test.py reuses the binning."""
    h = np.asarray(ins["h"], np.float32)
    src = np.asarray(ins["src"], np.int32)
    dst = np.asarray(ins["dst"], np.int32)
    D, C, NBLK = cfg["D"], cfg["C"], cfg["NBLK"]
    CP = 128
    KD = D // 128

    pp = _prep(h, src, dst, cfg)
    use_bias = bool(np.any(ins["b1"]) or np.any(ins["b2"]) or np.any(ins["b3"]))
    _get_prog(cfg, pp["kt_blk"], use_bias)

    w12 = np.stack([
        np.asarray(ins["W1"], np.float32).reshape(KD, 128, D).transpose(1, 0, 2),
        np.asarray(ins["W2"], np.float32).reshape(KD, 128, D).transpose(1, 0, 2)])
    w3 = np.zeros((128, KD, CP), np.float32)
    w3[:, :, :C] = np.asarray(ins["W3"], np.float32).reshape(KD, 128, C) \
        .transpose(1, 0, 2)
    biases = np.concatenate([
        np.asarray(ins["b1"], np.float32),
        np.asarray(ins["b2"], np.float32),
        np.asarray(ins["b3"], np.float32), np.zeros(CP - C, np.float32),
        np.ones(128, np.float32)])[None, :]
    ident = np.eye(128, dtype=np.float32)

    in_maps = [
        dict(hx=h.astype(BF16), sker=np.ascontiguousarray(pp["S"][c]).astype(BF16),
             idx1=pp["idx1"][c], idx23=pp["idx23"][c],
             w12=w12.astype(BF16), w3=w3.astype(BF16),
             ident=ident.astype(BF16), biases=biases.astype(BF16))
        for c in range(N_CORES)
    ]
    return in_maps, pp


def kernel(h, src, dst, W1, b1, W2, b2, W3, b3, cfg=CFG_FULL):
    from concourse.bass_utils import run_bass_kernel_spmd

    ins = dict(h=h, src=src, dst=dst, W1=W1, b1=b1, W2=W2, b2=b2,
               W3=W3, b3=b3)
    in_maps, pp = _in_maps(ins, cfg)
    nc = _LAST_NC[0]
    res = run_bass_kernel_spmd(nc, in_maps, core_ids=list(range(N_CORES)))

    N, C = cfg["N"], cfg["C"]
    out = np.zeros((N, C), np.float32)
    rows = pp["row_of_node"]
    allout = np.concatenate([res.results[c]["out"] for c in range(N_CORES)],
                            axis=0)
    out[:, :] = allout[rows]
    return out
